# revision 31
# baseline (speedup 1.0000x reference)
"""Bass/Trainium2 kernel for nn_HadamardClassifier.

Math: out = -scale * l2norm(x) @ H + bias, with H = H_16384[:2048, :14951]
(Sylvester). Since H_16384 = H_8 (x) H_2048 and rows < 2048 hit only row 0 of
the H_8 factor (all +1), H is just H_2048 tiled horizontally:
    out[:, j] = (x * (-scale/||x||_2)) @ H_2048[:, j % 2048] + bias[j]

Sharding: batch-parallel across 8 cores (512 rows each).

Numerics (gate is rel_err < 2e-2 vs max|out|): H_2048 entries are exactly
+-1 so bf16 H is lossless; raw x is rounded to bf16 (single matmul pass,
f32 PSUM accumulate) giving ~1e-3 rel err; the row factor -scale/||x|| is
applied per-partition during the PSUM->SBUF copy; output is stored as fp16
(~3e-4 additional) and upcast to f32 on the host after the gather.

Engine plan per core: sync/HWDGE does all HBM traffic (loads up front,
then one big out-DMA per (js, cb)); PE transposes raw x then streams 256
matmuls; ACT does norms + scaled PSUM->fp16 copies; DVE does the bias adds
as one stride-0-broadcast [128, 8, 512] op per (js, cb); GpSimd only does
4 wide bias partition-broadcasts.
"""

import math

import numpy as np

B, IN, OUT = 4096, 2048, 14951
NCORES = 8
BLOC = B // NCORES  # 512
P = 128
PERIOD = 2048
NFULL = 7  # full 2048-wide output blocks
TAIL = OUT - NFULL * PERIOD  # 615
EPS = 1e-12
NCB = BLOC // P  # 4 batch chunks per core
NIC = IN // P  # 16 contraction chunks
NJS = PERIOD // 512  # 4 column slabs of 512
# bias packed js-major: per js, its 512-col piece of every block, 512-padded
NBLK_JS = [8, 8, 7, 7]  # js1's 8th seg is the 103-wide tail (padded)
OFF_JS = [0, 4096, 8192, 11776]
BIAS_PACK = 15360
TAILW = [512, 103, 0, 0]  # tail-block width per js

_CACHE = {}
LAST_RESULT = None
PROFILE = False


def _build(scale_val: float):
    from contextlib import ExitStack

    import concourse.bass as bass
    import concourse.mybir as mybir
    import concourse.tile as tile
    from concourse import bacc, masks

    f32 = mybir.dt.float32
    bf16 = mybir.dt.bfloat16
    f16 = mybir.dt.float16
    nc = bacc.Bacc("TRN2", target_bir_lowering=False, debug=False,
                   num_devices=NCORES)

    x_d = nc.dram_tensor("x", [BLOC, IN], f32, kind="ExternalInput")
    h_d = nc.dram_tensor("h", [IN, PERIOD], bf16, kind="ExternalInput")
    b_d = nc.dram_tensor("bias", [1, BIAS_PACK], f16, kind="ExternalInput")
    o_d = nc.dram_tensor("out", [BLOC, OUT], f16, kind="ExternalOutput")

    # [2048 rows] -> [p, ic] view so each SBUF partition p holds rows ic*128+p
    h_v = h_d[:, :].rearrange("(ic p) j -> p ic j", p=P)
    # blocks 0..6 of the output as [rows, blk, col-in-block]
    o_main = o_d[:, 0 : NFULL * PERIOD].rearrange("r (blk c) -> r blk c",
                                                  c=PERIOD)

    with tile.TileContext(nc) as tc, ExitStack() as ctx:
        p_const = ctx.enter_context(tc.tile_pool(name="const", bufs=1))
        p_x = ctx.enter_context(tc.tile_pool(name="xload", bufs=NCB))
        p_w = ctx.enter_context(tc.tile_pool(name="work", bufs=1))
        p_ss = ctx.enter_context(tc.tile_pool(name="small", bufs=16))
        p_xth = ctx.enter_context(tc.tile_pool(name="xth", bufs=NCB))
        p_h = ctx.enter_context(tc.tile_pool(name="hslab", bufs=8))
        p_z = ctx.enter_context(tc.tile_pool(name="zsb", bufs=4))
        p_o = ctx.enter_context(tc.tile_pool(name="ostage", bufs=5))
        p_pst = ctx.enter_context(
            tc.tile_pool(name="psum_t", bufs=2, space="PSUM"))
        p_psz = ctx.enter_context(
            tc.tile_pool(name="psum_z", bufs=4, space="PSUM"))
        p_psw = ctx.enter_context(
            tc.tile_pool(name="psum_w", bufs=2, space="PSUM"))

        # HAM management: the NC clock gate throttles to 4/8 (halving PE,
        # DVE, Q7 AND DMA-descriptor speed) unless the PE runs wide matmul
        # streams; narrow ops (transposes) don't register. Open the gate
        # with a contiguous junk-matmul block that depends only on a memset,
        # and keep it open through PE-light stretches with more of them.
        junk512 = p_const.tile([P, 512], bf16, tag="junk512")
        nc.gpsimd.memset(junk512[:], 0.0)

        def keep_warm_m(n):
            for _ in range(n):
                pb = p_psw.tile([P, 512], f32, tag="warmM")
                nc.tensor.matmul(pb[:], junk512[:, 0:P], junk512[:],
                                 start=True, stop=True)

        keep_warm_m(20)

        ident = p_const.tile([P, P], f32, tag="ident")
        masks.make_identity(nc, ident[:])

        # ---- all HBM loads up front on the sync/HWDGE queue, in priority
        # order: bias row, x chunks + js0's H, then the rest of H
        bias_rep = p_const.tile([P, BIAS_PACK], f16, tag="bias_rep")
        nc.sync.dma_start(out=bias_rep[0:1, :], in_=b_d[:, :])

        xnats = []
        hq_tiles = {}

        def load_h(js, half):
            hq = p_h.tile([P, 8, 512], bf16, tag="hslab")
            nc.sync.dma_start(
                out=hq[:],
                in_=h_v[:, half * 8 : half * 8 + 8,
                        js * 512 : js * 512 + 512])
            hq_tiles[(js, half)] = hq

        for cb in range(NCB):
            xnat = p_x.tile([P, IN], f32, tag="xnat")
            nc.sync.dma_start(out=xnat[:], in_=x_d[cb * P : (cb + 1) * P, :])
            xnats.append(xnat)
            if cb < 2:
                load_h(0, cb)  # js0's two halves right after x0/x1
        for js in range(1, NJS):
            for half in range(2):
                load_h(js, half)

        # ---- bias replication: 512-wide partition-broadcast chunks on
        # gpsimd (its queue is otherwise idle), in js order so each js
        # segment beats its adds by a wide margin
        for js in range(NJS):
            o0 = OFF_JS[js]
            for blk in range(NBLK_JS[js]):
                a = o0 + blk * 512
                nc.gpsimd.partition_broadcast(bias_rep[:, a : a + 512],
                                              bias_rep[0:1, a : a + 512])

        # ---- phase 1: per chunk, row norms (ACT+DVE) and raw-x transposes
        # (PE); the -scale/||x|| factor is applied later on the psz->zsb copy
        mults = []
        xths = []
        for cb in range(NCB):
            xnat = xnats[cb]
            sq = p_w.tile([P, IN], bf16, tag="work")
            ss = p_ss.tile([P, 1], f32, tag="ss")
            nc.scalar.activation(sq[:], xnat[:],
                                 mybir.ActivationFunctionType.Square,
                                 accum_out=ss[:])
            nc.vector.tensor_scalar_max(ss[:], ss[:], EPS)
            nrm = p_ss.tile([P, 1], f32, tag="nrm")
            nc.scalar.sqrt(nrm[:], ss[:])
            inv = p_ss.tile([P, 1], f32, tag="inv")
            nc.vector.reciprocal(inv[:], nrm[:])
            mult = p_ss.tile([P, 1], f32, tag="mult")
            nc.vector.tensor_scalar_mul(mult[:], inv[:], -scale_val)
            mults.append(mult)

            xth = p_xth.tile([P, NIC, P], bf16, tag="xth")
            for ic in range(NIC):
                pst = p_pst.tile([P, P], f32, tag="pst")
                nc.tensor.transpose(pst[:], xnat[:, ic * P : (ic + 1) * P],
                                    ident[:])
                nc.vector.tensor_copy(xth[:, ic, :], pst[:])
            keep_warm_m(4)
            xths.append(xth)

        # ---- phase 2: Z = x' @ H_2048 slab by slab; scale, add bias; store
        for js in range(NJS):
            c0 = js * 512
            boff = OFF_JS[js]
            nblk = NBLK_JS[js]
            bseg = bias_rep[:, boff : boff + nblk * 512].rearrange(
                "p (b c) -> p b c", c=512)
            for cb in range(NCB):
                psz = p_psz.tile([P, 512], f32, tag="psz")
                for ic in range(NIC):
                    hap = hq_tiles[(js, ic // 8)][:, ic % 8, :]
                    nc.tensor.matmul(psz[:], xths[cb][:, ic, :], hap,
                                     start=(ic == 0), stop=(ic == NIC - 1))
                # psz * (-scale/||x||) -> fp16, per-partition scale on ACT
                zsb = p_z.tile([P, 1, 512], f16, tag="zsb")
                nc.scalar.mul(zsb[:, 0, :], psz[:], mults[cb][:, 0:1])

                ost = p_o.tile([P, 8, 512], f16, tag="ostage")
                nc.vector.tensor_add(
                    ost[:, 0:nblk, :],
                    zsb[:].to_broadcast([P, nblk, 512]),
                    bseg)

                r0 = cb * P
                nc.sync.dma_start(
                    out=o_main[r0 : r0 + P, 0:NFULL, c0 : c0 + 512],
                    in_=ost[:, 0:NFULL, :])
                tw = TAILW[js]
                if tw:
                    nc.sync.dma_start(
                        out=o_d[r0 : r0 + P, NFULL * PERIOD + c0 :
                                NFULL * PERIOD + c0 + tw],
                        in_=ost[:, NFULL, 0:tw])

        # drain tail: adds + out-DMAs run a few us past the last matmul;
        # keep the clock gate open until they finish
        keep_warm_m(30)

    nc.compile()
    return nc


def _pack_bias(bias: np.ndarray) -> np.ndarray:
    pack = np.zeros((1, BIAS_PACK), dtype=np.float16)
    for js in range(NJS):
        for blk in range(NBLK_JS[js]):
            src0 = blk * PERIOD + js * 512
            seg = bias[src0 : src0 + 512]
            pack[0, OFF_JS[js] + blk * 512 : OFF_JS[js] + blk * 512 + len(seg)] = seg
    return pack


def kernel(x, hadamard, scale, bias):
    global LAST_RESULT
    import ml_dtypes
    from concourse.bass_utils import run_bass_kernel_spmd

    x = np.ascontiguousarray(np.asarray(x, dtype=np.float32))
    hadamard = np.asarray(hadamard, dtype=np.float32)
    bias = np.asarray(bias, dtype=np.float32)
    scale_val = float(np.asarray(scale).reshape(-1)[0])

    h2 = np.ascontiguousarray(hadamard[:, :PERIOD])
    # the whole kernel rests on the 2048-periodicity of the weight columns
    for k in range(1, NFULL):
        assert np.array_equal(hadamard[:, k * PERIOD : (k + 1) * PERIOD], h2), (
            "hadamard is not 2048-periodic; kernel assumption violated")
    assert np.array_equal(hadamard[:, NFULL * PERIOD :], h2[:, :TAIL])
    h2b = h2.astype(ml_dtypes.bfloat16)
    assert np.array_equal(h2b.astype(np.float32), h2), "H not bf16-exact"

    key = scale_val
    if key not in _CACHE:
        _CACHE[key] = _build(scale_val)
    nc = _CACHE[key]

    bias_pack = _pack_bias(bias)
    in_maps = [
        {"x": np.ascontiguousarray(x[c * BLOC : (c + 1) * BLOC]),
         "h": h2b, "bias": bias_pack}
        for c in range(NCORES)
    ]
    res = run_bass_kernel_spmd(nc, in_maps, list(range(NCORES)),
                               trace=PROFILE)
    LAST_RESULT = res
    out = np.concatenate([res.results[c]["out"] for c in range(NCORES)],
                         axis=0).astype(np.float32)
    return out


# revision 35
# speedup vs baseline: 1.1206x; 1.1206x over previous
"""Bass/Trainium2 kernel for nn_HadamardClassifier.

Math: out = -scale * l2norm(x) @ H + bias, with H = H_16384[:2048, :14951]
(Sylvester). Since H_16384 = H_8 (x) H_2048 and rows < 2048 hit only row 0 of
the H_8 factor (all +1), H is just H_2048 tiled horizontally:
    out[:, j] = (x * (-scale/||x||_2)) @ H_2048[:, j % 2048] + bias[j]

Sharding: batch-parallel across 8 cores (512 rows each).

Numerics (gate is rel_err < 2e-2 vs max|out|): H_2048 entries are exactly
+-1 so bf16 H is lossless; raw x is rounded to bf16 (single matmul pass,
f32 PSUM accumulate) giving ~1e-3 rel err; the row factor -scale/||x|| is
applied per-partition during the PSUM->SBUF copy; output is stored as fp16
(~3e-4 additional) and upcast to f32 on the host after the gather.

Engine plan per core: sync/HWDGE does all HBM traffic (loads up front,
then one big out-DMA per (js, cb)); PE transposes raw x then streams 256
matmuls; ACT does norms + scaled PSUM->fp16 copies; DVE does the bias adds
as one stride-0-broadcast [128, 8, 512] op per (js, cb); GpSimd only does
4 wide bias partition-broadcasts.
"""

import math

import numpy as np

B, IN, OUT = 4096, 2048, 14951
NCORES = 8
BLOC = B // NCORES  # 512
P = 128
PERIOD = 2048
NFULL = 7  # full 2048-wide output blocks
TAIL = OUT - NFULL * PERIOD  # 615
EPS = 1e-12
NCB = BLOC // P  # 4 batch chunks per core
NIC = IN // P  # 16 contraction chunks
NJS = PERIOD // 512  # 4 column slabs of 512
# bias packed js-major: per js, its 512-col piece of every block, 512-padded
NBLK_JS = [8, 8, 7, 7]  # js1's 8th seg is the 103-wide tail (padded)
OFF_JS = [0, 4096, 8192, 11776]
BIAS_PACK = 15360
TAILW = [512, 103, 0, 0]  # tail-block width per js

_CACHE = {}
LAST_RESULT = None
PROFILE = False


def _build(scale_val: float):
    from contextlib import ExitStack

    import concourse.bass as bass
    import concourse.mybir as mybir
    import concourse.tile as tile
    from concourse import bacc, masks

    f32 = mybir.dt.float32
    bf16 = mybir.dt.bfloat16
    f16 = mybir.dt.float16
    nc = bacc.Bacc("TRN2", target_bir_lowering=False, debug=False,
                   num_devices=NCORES)

    x_d = nc.dram_tensor("x", [BLOC, IN], f32, kind="ExternalInput")
    h_d = nc.dram_tensor("h", [IN, PERIOD], bf16, kind="ExternalInput")
    b_d = nc.dram_tensor("bias", [1, BIAS_PACK], f16, kind="ExternalInput")
    o_d = nc.dram_tensor("out", [BLOC, OUT], f16, kind="ExternalOutput")

    # [2048 rows] -> [p, ic] view so each SBUF partition p holds rows ic*128+p
    h_v = h_d[:, :].rearrange("(ic p) j -> p ic j", p=P)
    # blocks 0..6 of the output as [rows, blk, col-in-block]
    o_main = o_d[:, 0 : NFULL * PERIOD].rearrange("r (blk c) -> r blk c",
                                                  c=PERIOD)

    with tile.TileContext(nc) as tc, ExitStack() as ctx:
        p_const = ctx.enter_context(tc.tile_pool(name="const", bufs=1))
        p_x = ctx.enter_context(tc.tile_pool(name="xload", bufs=NCB))
        p_w = ctx.enter_context(tc.tile_pool(name="work", bufs=1))
        p_ss = ctx.enter_context(tc.tile_pool(name="small", bufs=16))
        p_xth = ctx.enter_context(tc.tile_pool(name="xth", bufs=NCB))
        p_h = ctx.enter_context(tc.tile_pool(name="hslab", bufs=8))
        p_z = ctx.enter_context(tc.tile_pool(name="zsb", bufs=4))
        p_o = ctx.enter_context(tc.tile_pool(name="ostage", bufs=3))
        p_pst = ctx.enter_context(
            tc.tile_pool(name="psum_t", bufs=2, space="PSUM"))
        p_psz = ctx.enter_context(
            tc.tile_pool(name="psum_z", bufs=4, space="PSUM"))
        p_psw = ctx.enter_context(
            tc.tile_pool(name="psum_w", bufs=2, space="PSUM"))

        # HAM management: the NC clock gate throttles to 4/8 (halving PE,
        # DVE, Q7 AND DMA-descriptor speed) unless the PE runs wide matmul
        # streams; narrow ops (transposes) don't register. Open the gate
        # with a contiguous junk-matmul block that depends only on a memset,
        # and keep it open through PE-light stretches with more of them.
        junk512 = p_const.tile([P, 512], bf16, tag="junk512")
        nc.gpsimd.memset(junk512[:], 0.0)

        def keep_warm_m(n):
            for _ in range(n):
                pb = p_psw.tile([P, 512], f32, tag="warmM")
                nc.tensor.matmul(pb[:], junk512[:, 0:P], junk512[:],
                                 start=True, stop=True)

        keep_warm_m(20)

        ident = p_const.tile([P, P], f32, tag="ident")
        masks.make_identity(nc, ident[:])

        # ---- all HBM loads up front on the sync/HWDGE queue, in priority
        # order: bias row, x chunks + js0's H, then the rest of H
        bias_rep = p_const.tile([P, BIAS_PACK], f16, tag="bias_rep")
        nc.sync.dma_start(out=bias_rep[0:1, :], in_=b_d[:, :])

        xnats = []
        hq_tiles = {}

        def load_h(js, half):
            hq = p_h.tile([P, 8, 512], bf16, tag="hslab")
            nc.sync.dma_start(
                out=hq[:],
                in_=h_v[:, half * 8 : half * 8 + 8,
                        js * 512 : js * 512 + 512])
            hq_tiles[(js, half)] = hq

        for cb in range(NCB):
            xnat = p_x.tile([P, IN], f32, tag="xnat")
            nc.sync.dma_start(out=xnat[:], in_=x_d[cb * P : (cb + 1) * P, :])
            xnats.append(xnat)
            if cb < 2:
                load_h(0, cb)  # js0's two halves right after x0/x1
        for js in range(1, NJS):
            for half in range(2):
                load_h(js, half)

        # ---- bias replication: 512-wide partition-broadcast chunks on
        # gpsimd. js0/js1 up front; js2/js3 dribbled into phase 2 so Q7's
        # out-DMAs aren't stuck behind them
        def bcast_chunks(chunks):
            for js, blk in chunks:
                a = OFF_JS[js] + blk * 512
                nc.gpsimd.partition_broadcast(bias_rep[:, a : a + 512],
                                              bias_rep[0:1, a : a + 512])

        bcast_chunks([(js, blk) for js in (0, 1)
                      for blk in range(NBLK_JS[js])])
        bcast_rest = [(js, blk) for js in (2, 3)
                      for blk in range(NBLK_JS[js])]

        # ---- phase 1: per chunk, row norms (ACT+DVE) and raw-x transposes
        # (PE); the -scale/||x|| factor is applied later on the psz->zsb copy
        mults = []
        xths = []
        for cb in range(NCB):
            xnat = xnats[cb]
            sq = p_w.tile([P, IN], bf16, tag="work")
            ss = p_ss.tile([P, 1], f32, tag="ss")
            nc.scalar.activation(sq[:], xnat[:],
                                 mybir.ActivationFunctionType.Square,
                                 accum_out=ss[:])
            nc.vector.tensor_scalar_max(ss[:], ss[:], EPS)
            nrm = p_ss.tile([P, 1], f32, tag="nrm")
            nc.scalar.sqrt(nrm[:], ss[:])
            inv = p_ss.tile([P, 1], f32, tag="inv")
            nc.vector.reciprocal(inv[:], nrm[:])
            mult = p_ss.tile([P, 1], f32, tag="mult")
            nc.vector.tensor_scalar_mul(mult[:], inv[:], -scale_val)
            mults.append(mult)

            xth = p_xth.tile([P, NIC, P], bf16, tag="xth")
            for ic in range(NIC):
                pst = p_pst.tile([P, P], f32, tag="pst")
                nc.tensor.transpose(pst[:], xnat[:, ic * P : (ic + 1) * P],
                                    ident[:])
                nc.vector.tensor_copy(xth[:, ic, :], pst[:])
            keep_warm_m(4)
            xths.append(xth)

        # ---- phase 2: Z = x' @ H_2048, two 512-col slabs (js-pair) per
        # iteration staged into one [128, 8, 1024] tile -> ONE main out-DMA
        # per (jp, cb). Each DMA's end-of-transfer semaphore stalls its
        # queue on an HBM write-receipt round-trip (~1-1.5us dead time), so
        # mains are both halved in count and alternated between the sync
        # HWDGE ring and the gpsimd SWDGE ring so the stalls overlap.
        for jp in range(2):
            for cb in range(NCB):
                ost = p_o.tile([P, 8, 1024], f16, tag="ostage")
                for jh in range(2):
                    js = 2 * jp + jh
                    boff = OFF_JS[js]
                    nblk = NBLK_JS[js]
                    bseg = bias_rep[:, boff : boff + nblk * 512].rearrange(
                        "p (b c) -> p b c", c=512)
                    psz = p_psz.tile([P, 512], f32, tag="psz")
                    for ic in range(NIC):
                        hap = hq_tiles[(js, ic // 8)][:, ic % 8, :]
                        nc.tensor.matmul(psz[:], xths[cb][:, ic, :], hap,
                                         start=(ic == 0),
                                         stop=(ic == NIC - 1))
                    # psz * (-scale/||x||) -> fp16 on ACT
                    zsb = p_z.tile([P, 1, 512], f16, tag="zsb")
                    nc.scalar.mul(zsb[:, 0, :], psz[:], mults[cb][:, 0:1])
                    nc.vector.tensor_add(
                        ost[:, 0:nblk, jh * 512 : jh * 512 + 512],
                        zsb[:].to_broadcast([P, nblk, 512]),
                        bseg)

                r0 = cb * P
                c0 = jp * 1024
                eng = nc.sync if cb % 2 == 0 else nc.gpsimd
                eng.dma_start(
                    out=o_main[r0 : r0 + P, 0:NFULL, c0 : c0 + 1024],
                    in_=ost[:, 0:NFULL, :])
                if jp == 0:
                    # blk-7 tail: js0's 512 + js1's 103 are contiguous in
                    # DRAM (cols 14336..14951) and in the staged tile
                    nc.sync.dma_start(
                        out=o_d[r0 : r0 + P, NFULL * PERIOD :
                                NFULL * PERIOD + TAIL],
                        in_=ost[:, NFULL, 0:TAIL])
                    # drip the js2/js3 bias replication behind Q7's DMAs
                    bcast_chunks(bcast_rest[cb * 4 : cb * 4 + 4])

        # drain tail: adds + out-DMAs run a few us past the last matmul;
        # keep the clock gate open until they finish
        keep_warm_m(30)

    nc.compile()
    return nc


def _pack_bias(bias: np.ndarray) -> np.ndarray:
    pack = np.zeros((1, BIAS_PACK), dtype=np.float16)
    for js in range(NJS):
        for blk in range(NBLK_JS[js]):
            src0 = blk * PERIOD + js * 512
            seg = bias[src0 : src0 + 512]
            pack[0, OFF_JS[js] + blk * 512 : OFF_JS[js] + blk * 512 + len(seg)] = seg
    return pack


def kernel(x, hadamard, scale, bias):
    global LAST_RESULT
    import ml_dtypes
    from concourse.bass_utils import run_bass_kernel_spmd

    x = np.ascontiguousarray(np.asarray(x, dtype=np.float32))
    hadamard = np.asarray(hadamard, dtype=np.float32)
    bias = np.asarray(bias, dtype=np.float32)
    scale_val = float(np.asarray(scale).reshape(-1)[0])

    h2 = np.ascontiguousarray(hadamard[:, :PERIOD])
    # the whole kernel rests on the 2048-periodicity of the weight columns
    for k in range(1, NFULL):
        assert np.array_equal(hadamard[:, k * PERIOD : (k + 1) * PERIOD], h2), (
            "hadamard is not 2048-periodic; kernel assumption violated")
    assert np.array_equal(hadamard[:, NFULL * PERIOD :], h2[:, :TAIL])
    h2b = h2.astype(ml_dtypes.bfloat16)
    assert np.array_equal(h2b.astype(np.float32), h2), "H not bf16-exact"

    key = scale_val
    if key not in _CACHE:
        _CACHE[key] = _build(scale_val)
    nc = _CACHE[key]

    bias_pack = _pack_bias(bias)
    in_maps = [
        {"x": np.ascontiguousarray(x[c * BLOC : (c + 1) * BLOC]),
         "h": h2b, "bias": bias_pack}
        for c in range(NCORES)
    ]
    res = run_bass_kernel_spmd(nc, in_maps, list(range(NCORES)),
                               trace=PROFILE)
    LAST_RESULT = res
    out = np.concatenate([res.results[c]["out"] for c in range(NCORES)],
                         axis=0).astype(np.float32)
    return out


# revision 36
# speedup vs baseline: 1.1821x; 1.0549x over previous
"""Bass/Trainium2 kernel for nn_HadamardClassifier.

Math: out = -scale * l2norm(x) @ H + bias, with H = H_16384[:2048, :14951]
(Sylvester). Two structure facts are exploited:
  1. H's columns are 2048-periodic: out[:, j] = Z[:, j %% 2048] + bias[j]
     with Z = xn @ H_2048.
  2. H_2048 = [[A, A], [A, -A]] with A = H_1024, so with
     u = x[:, :1024] + x[:, 1024:], v = x[:, :1024] - x[:, 1024:]:
     Z = [u @ A | v @ A] -- one FWHT butterfly level. Only the 2 MB A is
     loaded from HBM and matmul MACs halve.

Sharding: batch-parallel across 8 cores (512 rows each).

Numerics (gate is rel_err < 2e-2 vs max|out|): A entries are exactly +-1 so
bf16 is lossless; u/v are rounded to bf16 (f32 PSUM accumulate) ~1e-3 rel
err; the row factor -scale/||x|| is applied per-partition during the
PSUM->SBUF copy; output is stored as fp16 (~3e-4 additional) and upcast to
f32 on the host after the gather. Total ~1.6e-3.

Perf notes (hard-won):
  - The NC clock gate (HAM) throttles everything (PE, DVE, Q7, DMA
    descriptor processing) to ~half speed unless the PE sustains wide
    matmul streams; narrow ops don't register. So junk [128,128,512]
    matmuls pad the PE schedule wherever real PE work thinned out.
  - Each out-DMA's completion semaphore stalls its queue ~1-1.5us on an
    HBM write-receipt round-trip; mains are staged as [128, 8, 1024]
    (two 512-col js-slabs) and alternated sync/gpsimd to overlap stalls.
  - Q7 partition_broadcast is ~1us/512-chunk; js0/js1 replication runs up
    front, js2/js3 drips between Q7's out-DMAs.
"""

import math

import numpy as np

B, IN, OUT = 4096, 2048, 14951
NCORES = 8
BLOC = B // NCORES  # 512
P = 128
PERIOD = 2048
HALF = 1024
NFULL = 7  # full 2048-wide output blocks
TAIL = OUT - NFULL * PERIOD  # 615
EPS = 1e-12
NCB = BLOC // P  # 4 batch chunks per core
NIC = HALF // P  # 8 contraction chunks (over A's rows)
NJS = PERIOD // 512  # 4 column slabs of 512
# bias packed js-major: per js, its 512-col piece of every block, 512-padded
NBLK_JS = [8, 8, 7, 7]  # js1's 8th seg is the 103-wide tail (padded)
OFF_JS = [0, 4096, 8192, 11776]
BIAS_PACK = 15360

_CACHE = {}
LAST_RESULT = None
PROFILE = False


def _build(scale_val: float):
    from contextlib import ExitStack

    import concourse.bass as bass
    import concourse.mybir as mybir
    import concourse.tile as tile
    from concourse import bacc, masks

    f32 = mybir.dt.float32
    bf16 = mybir.dt.bfloat16
    f16 = mybir.dt.float16
    nc = bacc.Bacc("TRN2", target_bir_lowering=False, debug=False,
                   num_devices=NCORES)

    x_d = nc.dram_tensor("x", [BLOC, IN], f32, kind="ExternalInput")
    h_d = nc.dram_tensor("h", [HALF, HALF], bf16, kind="ExternalInput")
    b_d = nc.dram_tensor("bias", [1, BIAS_PACK], f16, kind="ExternalInput")
    o_d = nc.dram_tensor("out", [BLOC, OUT], f16, kind="ExternalOutput")

    # [1024 rows] -> [p, ic] view so each SBUF partition p holds rows ic*128+p
    h_v = h_d[:, :].rearrange("(ic p) j -> p ic j", p=P)
    # blocks 0..6 of the output as [rows, blk, col-in-block]
    o_main = o_d[:, 0 : NFULL * PERIOD].rearrange("r (blk c) -> r blk c",
                                                  c=PERIOD)

    with tile.TileContext(nc) as tc, ExitStack() as ctx:
        p_const = ctx.enter_context(tc.tile_pool(name="const", bufs=1))
        p_x = ctx.enter_context(tc.tile_pool(name="xload", bufs=NCB))
        p_uv = ctx.enter_context(tc.tile_pool(name="uv", bufs=2))
        p_w = ctx.enter_context(tc.tile_pool(name="work", bufs=1))
        p_ss = ctx.enter_context(tc.tile_pool(name="small", bufs=16))
        p_xth = ctx.enter_context(tc.tile_pool(name="xth", bufs=NCB))
        p_h = ctx.enter_context(tc.tile_pool(name="hslab", bufs=2))
        p_z = ctx.enter_context(tc.tile_pool(name="zsb", bufs=4))
        p_o = ctx.enter_context(tc.tile_pool(name="ostage", bufs=3))
        p_pst = ctx.enter_context(
            tc.tile_pool(name="psum_t", bufs=2, space="PSUM"))
        p_psz = ctx.enter_context(
            tc.tile_pool(name="psum_z", bufs=4, space="PSUM"))
        p_psw = ctx.enter_context(
            tc.tile_pool(name="psum_w", bufs=2, space="PSUM"))

        # junk-matmul machinery: opens the HAM clock gate at t~1 (depends
        # only on a memset) and pads the PE schedule everywhere else
        junk512 = p_const.tile([P, 512], bf16, tag="junk512")
        nc.gpsimd.memset(junk512[:], 0.0)

        def keep_warm_m(n):
            for _ in range(n):
                pb = p_psw.tile([P, 512], f32, tag="warmM")
                nc.tensor.matmul(pb[:], junk512[:, 0:P], junk512[:],
                                 start=True, stop=True)

        keep_warm_m(20)

        identb = p_const.tile([P, P], bf16, tag="identb")
        masks.make_identity(nc, identb[:])

        # ---- all HBM loads up front on the sync/HWDGE queue
        bias_rep = p_const.tile([P, BIAS_PACK], f16, tag="bias_rep")
        nc.sync.dma_start(out=bias_rep[0:1, :], in_=b_d[:, :])

        xnats = []
        hq_tiles = {}

        def load_h(half):
            hq = p_h.tile([P, NIC, 512], bf16, tag="hslab")
            nc.sync.dma_start(
                out=hq[:], in_=h_v[:, :, half * 512 : half * 512 + 512])
            hq_tiles[half] = hq

        for cb in range(NCB):
            xnat = p_x.tile([P, IN], f32, tag="xnat")
            nc.sync.dma_start(out=xnat[:], in_=x_d[cb * P : (cb + 1) * P, :])
            xnats.append(xnat)
            if cb < 2:
                load_h(cb)

        # ---- bias replication on gpsimd: js0/js1 up front, js2/js3
        # dripped between Q7's out-DMAs in phase 2
        def bcast_chunks(chunks):
            for js, blk in chunks:
                a = OFF_JS[js] + blk * 512
                nc.gpsimd.partition_broadcast(bias_rep[:, a : a + 512],
                                              bias_rep[0:1, a : a + 512])

        bcast_chunks([(js, blk) for js in (0, 1)
                      for blk in range(NBLK_JS[js])])
        bcast_rest = [(js, blk) for js in (2, 3)
                      for blk in range(NBLK_JS[js])]

        # ---- phase 1: row norms (ACT+DVE), FWHT butterfly (DVE), u/v
        # transposes (PE, bf16) with PSUM->SBUF copies split ACT/DVE
        mults = []
        xths = []
        for cb in range(NCB):
            xnat = xnats[cb]
            sq = p_w.tile([P, IN], bf16, tag="work")
            ss = p_ss.tile([P, 1], f32, tag="ss")
            nc.scalar.activation(sq[:], xnat[:],
                                 mybir.ActivationFunctionType.Square,
                                 accum_out=ss[:])
            nc.vector.tensor_scalar_max(ss[:], ss[:], EPS)
            nrm = p_ss.tile([P, 1], f32, tag="nrm")
            nc.scalar.sqrt(nrm[:], ss[:])
            inv = p_ss.tile([P, 1], f32, tag="inv")
            nc.vector.reciprocal(inv[:], nrm[:])
            mult = p_ss.tile([P, 1], f32, tag="mult")
            nc.vector.tensor_scalar_mul(mult[:], inv[:], -scale_val)
            mults.append(mult)

            uv = p_uv.tile([P, 2, HALF], bf16, tag="uv")
            nc.vector.tensor_add(uv[:, 0, :], xnat[:, 0:HALF],
                                 xnat[:, HALF:IN])
            nc.vector.tensor_sub(uv[:, 1, :], xnat[:, 0:HALF],
                                 xnat[:, HALF:IN])
            keep_warm_m(2)

            xth = p_xth.tile([P, 2, NIC, P], bf16, tag="xth")
            for g in range(2):
                for ic in range(NIC):
                    pst = p_pst.tile([P, P], bf16, tag="pst")
                    nc.tensor.transpose(
                        pst[:], uv[:, g, ic * P : (ic + 1) * P], identb[:])
                    if ic % 2 == 0:
                        nc.scalar.copy(xth[:, g, ic, :], pst[:])
                    else:
                        nc.vector.tensor_copy(xth[:, g, ic, :], pst[:])
                keep_warm_m(2)
            xths.append(xth)

        # ---- phase 2: per (jp, cb), two 512-col js-slabs -> one
        # [128, 8, 1024] staged tile -> one main out-DMA (alternating
        # sync/gpsimd queues) + one tail DMA for jp0. Junk matmuls after
        # each real group restore v3-level PE duty for the HAM monitor.
        for jp in range(2):
            for cb in range(NCB):
                ost = p_o.tile([P, 8, 1024], f16, tag="ostage")
                for jh in range(2):
                    js = 2 * jp + jh
                    boff = OFF_JS[js]
                    nblk = NBLK_JS[js]
                    bseg = bias_rep[:, boff : boff + nblk * 512].rearrange(
                        "p (b c) -> p b c", c=512)
                    psz = p_psz.tile([P, 512], f32, tag="psz")
                    for ic in range(NIC):
                        nc.tensor.matmul(psz[:], xths[cb][:, jp, ic, :],
                                         hq_tiles[jh][:, ic, :],
                                         start=(ic == 0),
                                         stop=(ic == NIC - 1))
                    keep_warm_m(7)
                    # psz * (-scale/||x||) -> fp16 on ACT
                    zsb = p_z.tile([P, 1, 512], f16, tag="zsb")
                    nc.scalar.mul(zsb[:, 0, :], psz[:], mults[cb][:, 0:1])
                    nc.vector.tensor_add(
                        ost[:, 0:nblk, jh * 512 : jh * 512 + 512],
                        zsb[:].to_broadcast([P, nblk, 512]),
                        bseg)

                r0 = cb * P
                c0 = jp * 1024
                eng = nc.sync if cb % 2 == 0 else nc.gpsimd
                eng.dma_start(
                    out=o_main[r0 : r0 + P, 0:NFULL, c0 : c0 + 1024],
                    in_=ost[:, 0:NFULL, :])
                if jp == 0:
                    # blk-7 tail: js0's 512 + js1's 103 are contiguous in
                    # DRAM (cols 14336..14951) and in the staged tile
                    nc.sync.dma_start(
                        out=o_d[r0 : r0 + P, NFULL * PERIOD :
                                NFULL * PERIOD + TAIL],
                        in_=ost[:, NFULL, 0:TAIL])
                    bcast_chunks(bcast_rest[cb * 4 : cb * 4 + 4])

        # drain tail: adds + out-DMAs run past the last real matmul; keep
        # the clock gate open until they finish
        keep_warm_m(30)

    nc.compile()
    return nc


def _pack_bias(bias: np.ndarray) -> np.ndarray:
    pack = np.zeros((1, BIAS_PACK), dtype=np.float16)
    for js in range(NJS):
        for blk in range(NBLK_JS[js]):
            src0 = blk * PERIOD + js * 512
            seg = bias[src0 : src0 + 512]
            pack[0, OFF_JS[js] + blk * 512 : OFF_JS[js] + blk * 512 + len(seg)] = seg
    return pack


def kernel(x, hadamard, scale, bias):
    global LAST_RESULT
    import ml_dtypes
    from concourse.bass_utils import run_bass_kernel_spmd

    x = np.ascontiguousarray(np.asarray(x, dtype=np.float32))
    hadamard = np.asarray(hadamard, dtype=np.float32)
    bias = np.asarray(bias, dtype=np.float32)
    scale_val = float(np.asarray(scale).reshape(-1)[0])

    h2 = np.ascontiguousarray(hadamard[:, :PERIOD])
    # the whole kernel rests on the 2048-periodicity of the weight columns
    for k in range(1, NFULL):
        assert np.array_equal(hadamard[:, k * PERIOD : (k + 1) * PERIOD], h2), (
            "hadamard is not 2048-periodic; kernel assumption violated")
    assert np.array_equal(hadamard[:, NFULL * PERIOD :], h2[:, :TAIL])
    # ... and on the Sylvester block structure H_2048 = [[A, A], [A, -A]]
    A = h2[:HALF, :HALF]
    assert np.array_equal(h2[HALF:, :HALF], A)
    assert np.array_equal(h2[:HALF, HALF:], A)
    assert np.array_equal(h2[HALF:, HALF:], -A)
    Ab = A.astype(ml_dtypes.bfloat16)
    assert np.array_equal(Ab.astype(np.float32), A), "A not bf16-exact"

    key = scale_val
    if key not in _CACHE:
        _CACHE[key] = _build(scale_val)
    nc = _CACHE[key]

    bias_pack = _pack_bias(bias)
    in_maps = [
        {"x": np.ascontiguousarray(x[c * BLOC : (c + 1) * BLOC]),
         "h": Ab, "bias": bias_pack}
        for c in range(NCORES)
    ]
    res = run_bass_kernel_spmd(nc, in_maps, list(range(NCORES)),
                               trace=PROFILE)
    LAST_RESULT = res
    out = np.concatenate([res.results[c]["out"] for c in range(NCORES)],
                         axis=0).astype(np.float32)
    return out


# revision 41
# speedup vs baseline: 1.2432x; 1.0517x over previous
"""Bass/Trainium2 kernel for nn_HadamardClassifier.

Math: out = -scale * l2norm(x) @ H + bias, with H = H_16384[:2048, :14951]
(Sylvester). Two structure facts are exploited:
  1. H's columns are 2048-periodic: out[:, j] = Z[:, j %% 2048] + bias[j]
     with Z = xn @ H_2048.
  2. H_2048 = [[A, A], [A, -A]] with A = H_1024, so with
     u = x[:, :1024] + x[:, 1024:], v = x[:, :1024] - x[:, 1024:]:
     Z = [u @ A | v @ A] -- one FWHT butterfly level. Only the 2 MB A is
     loaded from HBM and matmul MACs halve.

Sharding: batch-parallel across 8 cores (512 rows each).

Numerics (gate is rel_err < 2e-2 vs max|out|): A entries are exactly +-1 so
bf16 is lossless; u/v are rounded to bf16 (f32 PSUM accumulate) ~1e-3 rel
err; the row factor -scale/||x|| is applied per-partition during the
PSUM->SBUF copy; output is stored as fp16 (~3e-4 additional) and upcast to
f32 on the host after the gather. Total ~1.6e-3.

Perf notes (hard-won):
  - The NC clock gate (HAM) throttles everything (PE, DVE, Q7, DMA
    descriptor processing) to ~half speed unless the PE sustains wide
    matmul streams; narrow ops don't register. So junk [128,128,512]
    matmuls pad the PE schedule wherever real PE work thinned out.
  - Each out-DMA's completion semaphore stalls its queue ~1-1.5us on an
    HBM write-receipt round-trip; mains are staged as [128, 8, 1024]
    (two 512-col js-slabs) and alternated sync/gpsimd to overlap stalls.
  - Q7 partition_broadcast is ~1us/512-chunk; js0/js1 replication runs up
    front, js2/js3 drips between Q7's out-DMAs.
"""

import math

import numpy as np

B, IN, OUT = 4096, 2048, 14951
NCORES = 8
BLOC = B // NCORES  # 512
P = 128
PERIOD = 2048
HALF = 1024
NFULL = 7  # full 2048-wide output blocks
TAIL = OUT - NFULL * PERIOD  # 615
EPS = 1e-12
NCB = BLOC // P  # 4 batch chunks per core
NIC = HALF // P  # 8 contraction chunks (over A's rows)
NJS = PERIOD // 512  # 4 column slabs of 512
# bias packed js-major: per js, its 512-col piece of every block, 512-padded
NBLK_JS = [8, 8, 7, 7]  # js1's 8th seg is the 103-wide tail (padded)
OFF_JS = [0, 4096, 8192, 11776]
BIAS_PACK = 15360

_CACHE = {}
LAST_RESULT = None
PROFILE = False


def _build(scale_val: float):
    from contextlib import ExitStack

    import concourse.bass as bass
    import concourse.mybir as mybir
    import concourse.tile as tile
    from concourse import bacc, masks

    f32 = mybir.dt.float32
    bf16 = mybir.dt.bfloat16
    f16 = mybir.dt.float16
    nc = bacc.Bacc("TRN2", target_bir_lowering=False, debug=False,
                   num_devices=NCORES)

    x_d = nc.dram_tensor("x", [BLOC, IN], f32, kind="ExternalInput")
    h_d = nc.dram_tensor("h", [HALF, HALF], bf16, kind="ExternalInput")
    b_d = nc.dram_tensor("bias", [1, BIAS_PACK], f16, kind="ExternalInput")
    o_d = nc.dram_tensor("out", [BLOC, OUT], f16, kind="ExternalOutput")

    # [1024 rows] -> [p, ic] view so each SBUF partition p holds rows ic*128+p
    h_v = h_d[:, :].rearrange("(ic p) j -> p ic j", p=P)
    # blocks 0..6 of the output as [rows, blk, col-in-block]
    o_main = o_d[:, 0 : NFULL * PERIOD].rearrange("r (blk c) -> r blk c",
                                                  c=PERIOD)

    with tile.TileContext(nc) as tc, ExitStack() as ctx:
        p_const = ctx.enter_context(tc.tile_pool(name="const", bufs=1))
        p_x = ctx.enter_context(tc.tile_pool(name="xload", bufs=NCB))
        p_uv = ctx.enter_context(tc.tile_pool(name="uv", bufs=2))
        p_w = ctx.enter_context(tc.tile_pool(name="work", bufs=1))
        p_ss = ctx.enter_context(tc.tile_pool(name="small", bufs=16))
        p_xth = ctx.enter_context(tc.tile_pool(name="xth", bufs=NCB))
        p_h = ctx.enter_context(tc.tile_pool(name="hslab", bufs=2))
        p_z = ctx.enter_context(tc.tile_pool(name="zsb", bufs=4))
        p_o = ctx.enter_context(tc.tile_pool(name="ostage", bufs=3))
        p_pst = ctx.enter_context(
            tc.tile_pool(name="psum_t", bufs=2, space="PSUM"))
        p_psz = ctx.enter_context(
            tc.tile_pool(name="psum_z", bufs=4, space="PSUM"))
        p_psw = ctx.enter_context(
            tc.tile_pool(name="psum_w", bufs=2, space="PSUM"))

        # junk-matmul machinery: opens the HAM clock gate at t~1 (depends
        # only on a memset) and pads the PE schedule everywhere else
        junk512 = p_const.tile([P, 512], bf16, tag="junk512")
        nc.gpsimd.memset(junk512[:], 0.0)
        onesb = p_const.tile([1, P], f16, tag="onesb")
        nc.gpsimd.memset(onesb[:], 1.0)

        def keep_warm_m(n):
            for _ in range(n):
                pb = p_psw.tile([P, 512], f32, tag="warmM")
                nc.tensor.matmul(pb[:], junk512[:, 0:P], junk512[:],
                                 start=True, stop=True)

        keep_warm_m(20)

        identb = p_const.tile([P, P], bf16, tag="identb")
        masks.make_identity(nc, identb[:])

        # ---- all HBM loads up front on the sync/HWDGE queue
        bias_rep = p_const.tile([P, BIAS_PACK], f16, tag="bias_rep")
        nc.sync.dma_start(out=bias_rep[0:1, :], in_=b_d[:, :])

        xnats = []
        hq_tiles = {}

        def load_h(half):
            hq = p_h.tile([P, NIC, 512], bf16, tag="hslab")
            nc.sync.dma_start(
                out=hq[:], in_=h_v[:, :, half * 512 : half * 512 + 512])
            hq_tiles[half] = hq

        for cb in range(NCB):
            xnat = p_x.tile([P, IN], f32, tag="xnat")
            nc.sync.dma_start(out=xnat[:], in_=x_d[cb * P : (cb + 1) * P, :])
            xnats.append(xnat)
            if cb < 2:
                load_h(cb)

        # ---- bias replication, split three ways so both js0 and js1 are
        # ready by ~14us (the first out-pair needs BOTH): js0 on gpsimd
        # partition_broadcast (8 chunks up front), js1 on the PE as
        # ones^T (x) bias_row K=1 matmuls with ACT/DVE copies, js2/js3
        # dripped between Q7's phase-2 out-DMAs
        def bcast_chunks(chunks):
            for js, blk in chunks:
                a = OFF_JS[js] + blk * 512
                nc.gpsimd.partition_broadcast(bias_rep[:, a : a + 512],
                                              bias_rep[0:1, a : a + 512])

        bcast_chunks([(0, blk) for blk in range(NBLK_JS[0])])
        bcast_rest = [(js, blk) for js in (2, 3)
                      for blk in range(NBLK_JS[js])]

        for blk in range(NBLK_JS[1]):
            a = OFF_JS[1] + blk * 512
            pb = p_psw.tile([P, 512], f32, tag="warmM")
            nc.tensor.matmul(pb[:], onesb[:], bias_rep[0:1, a : a + 512],
                             start=True, stop=True)
            if blk % 2 == 0:
                nc.scalar.copy(bias_rep[:, a : a + 512], pb[:])
            else:
                nc.vector.tensor_copy(bias_rep[:, a : a + 512], pb[:])

        # ---- phase 1: row norms (ACT+DVE), FWHT butterfly (DVE), u/v
        # transposes (PE, bf16) with PSUM->SBUF copies split ACT/DVE
        mults = []
        xths = []
        for cb in range(NCB):
            xnat = xnats[cb]
            sq = p_w.tile([P, IN], bf16, tag="work")
            ss = p_ss.tile([P, 1], f32, tag="ss")
            nc.scalar.activation(sq[:], xnat[:],
                                 mybir.ActivationFunctionType.Square,
                                 accum_out=ss[:])
            nc.vector.tensor_scalar_max(ss[:], ss[:], EPS)
            nrm = p_ss.tile([P, 1], f32, tag="nrm")
            nc.scalar.sqrt(nrm[:], ss[:])
            inv = p_ss.tile([P, 1], f32, tag="inv")
            nc.vector.reciprocal(inv[:], nrm[:])
            mult = p_ss.tile([P, 1], f32, tag="mult")
            nc.vector.tensor_scalar_mul(mult[:], inv[:], -scale_val)
            mults.append(mult)

            uv = p_uv.tile([P, 2, HALF], bf16, tag="uv")
            nc.vector.tensor_add(uv[:, 0, :], xnat[:, 0:HALF],
                                 xnat[:, HALF:IN])
            nc.vector.tensor_sub(uv[:, 1, :], xnat[:, 0:HALF],
                                 xnat[:, HALF:IN])
            keep_warm_m(2)

            xth = p_xth.tile([P, 2, NIC, P], bf16, tag="xth")
            for g in range(2):
                for ic in range(NIC):
                    pst = p_pst.tile([P, P], bf16, tag="pst")
                    nc.tensor.transpose(
                        pst[:], uv[:, g, ic * P : (ic + 1) * P], identb[:])
                    if ic % 2 == 0:
                        nc.scalar.copy(xth[:, g, ic, :], pst[:])
                    else:
                        nc.vector.tensor_copy(xth[:, g, ic, :], pst[:])
                keep_warm_m(2)
            xths.append(xth)

        # ---- phase 2: per (jp, cb), two 512-col js-slabs -> one
        # [128, 8, 1024] staged tile -> one main out-DMA (alternating
        # sync/gpsimd queues) + one tail DMA for jp0. Junk matmuls after
        # each real group restore v3-level PE duty for the HAM monitor.
        for jp in range(2):
            for cb in range(NCB):
                ost = p_o.tile([P, 8, 1024], f16, tag="ostage")
                for jh in range(2):
                    js = 2 * jp + jh
                    boff = OFF_JS[js]
                    nblk = NBLK_JS[js]
                    bseg = bias_rep[:, boff : boff + nblk * 512].rearrange(
                        "p (b c) -> p b c", c=512)
                    psz = p_psz.tile([P, 512], f32, tag="psz")
                    for ic in range(NIC):
                        nc.tensor.matmul(psz[:], xths[cb][:, jp, ic, :],
                                         hq_tiles[jh][:, ic, :],
                                         start=(ic == 0),
                                         stop=(ic == NIC - 1))
                    keep_warm_m(3)
                    # psz * (-scale/||x||) -> fp16 on ACT
                    zsb = p_z.tile([P, 1, 512], f16, tag="zsb")
                    nc.scalar.mul(zsb[:, 0, :], psz[:], mults[cb][:, 0:1])
                    nc.vector.tensor_add(
                        ost[:, 0:nblk, jh * 512 : jh * 512 + 512],
                        zsb[:].to_broadcast([P, nblk, 512]),
                        bseg)

                r0 = cb * P
                c0 = jp * 1024
                eng = nc.sync if cb % 2 == 0 else nc.gpsimd
                eng.dma_start(
                    out=o_main[r0 : r0 + P, 0:NFULL, c0 : c0 + 1024],
                    in_=ost[:, 0:NFULL, :])
                if jp == 0:
                    # blk-7 tail: js0's 512 + js1's 103 are contiguous in
                    # DRAM (cols 14336..14951) and in the staged tile
                    nc.sync.dma_start(
                        out=o_d[r0 : r0 + P, NFULL * PERIOD :
                                NFULL * PERIOD + TAIL],
                        in_=ost[:, NFULL, 0:TAIL])
                    bcast_chunks(bcast_rest[cb * 4 : cb * 4 + 4])

        # drain tail: adds + out-DMAs run past the last real matmul; keep
        # the clock gate open until they finish
        keep_warm_m(45)

    nc.compile()
    return nc


def _pack_bias(bias: np.ndarray) -> np.ndarray:
    pack = np.zeros((1, BIAS_PACK), dtype=np.float16)
    for js in range(NJS):
        for blk in range(NBLK_JS[js]):
            src0 = blk * PERIOD + js * 512
            seg = bias[src0 : src0 + 512]
            pack[0, OFF_JS[js] + blk * 512 : OFF_JS[js] + blk * 512 + len(seg)] = seg
    return pack


def kernel(x, hadamard, scale, bias):
    global LAST_RESULT
    import ml_dtypes
    from concourse.bass_utils import run_bass_kernel_spmd

    x = np.ascontiguousarray(np.asarray(x, dtype=np.float32))
    hadamard = np.asarray(hadamard, dtype=np.float32)
    bias = np.asarray(bias, dtype=np.float32)
    scale_val = float(np.asarray(scale).reshape(-1)[0])

    h2 = np.ascontiguousarray(hadamard[:, :PERIOD])
    # the whole kernel rests on the 2048-periodicity of the weight columns
    for k in range(1, NFULL):
        assert np.array_equal(hadamard[:, k * PERIOD : (k + 1) * PERIOD], h2), (
            "hadamard is not 2048-periodic; kernel assumption violated")
    assert np.array_equal(hadamard[:, NFULL * PERIOD :], h2[:, :TAIL])
    # ... and on the Sylvester block structure H_2048 = [[A, A], [A, -A]]
    A = h2[:HALF, :HALF]
    assert np.array_equal(h2[HALF:, :HALF], A)
    assert np.array_equal(h2[:HALF, HALF:], A)
    assert np.array_equal(h2[HALF:, HALF:], -A)
    Ab = A.astype(ml_dtypes.bfloat16)
    assert np.array_equal(Ab.astype(np.float32), A), "A not bf16-exact"

    key = scale_val
    if key not in _CACHE:
        _CACHE[key] = _build(scale_val)
    nc = _CACHE[key]

    bias_pack = _pack_bias(bias)
    in_maps = [
        {"x": np.ascontiguousarray(x[c * BLOC : (c + 1) * BLOC]),
         "h": Ab, "bias": bias_pack}
        for c in range(NCORES)
    ]
    res = run_bass_kernel_spmd(nc, in_maps, list(range(NCORES)),
                               trace=PROFILE)
    LAST_RESULT = res
    out = np.concatenate([res.results[c]["out"] for c in range(NCORES)],
                         axis=0).astype(np.float32)
    return out
